# revision 25
# baseline (speedup 1.0000x reference)
"""Trainium2 Bass kernel for nn_Net_16174846837292 (NNConv GNN message passing).

Strategy (graph-sharded, aggregation-folded, single fp16 a2 pass):
  pooled[g,o] = sum_{e: batch[dst[e]]=g} w_e * msg[e,o],  w_e = 1/max(cnt[dst_e],1)
  msg[e,o]    = sum_{k,i} e3[e,k]*h[src_e,i]*e4w[k,i*128+o] + sum_i h[src_e,i]*e4b[i*128+o]
  => pooled[g,o] = sum_k ZG_g[:,k]^T A2f[:,k*128+o] + HW_g^T Br
     ZG_g[i,k] = sum_{e in g} (w_e h[src_e,i]) e3[e,k],  HW_g[i] = sum_e w_e h[src_e,i]

Sharding: graphs are greedily re-balanced across the 8 cores by edge count
(the host owns the graph->core map and reassembles the output), 8 graph
slots per core sorted largest-first, so the shared compiled capacity
profile (max over cores per slot, 64-aligned) stays tight: EP ~1280 vs the
1536 a fixed worst-case cap needs. In-degree weights are per-edge host
constants and NO collectives are needed. The program is compiled per
capacity-profile (cached), so any input re-specializes correctly.

Schedule (from perfetto traces of the 38.4us baseline):
- the 4MB a2 stream starts at the first possible DMA slot, split into
  8 x 512KB chunks: evens on the sync HWDGE ring FIFO-behind the small
  inputs, odds on the SWDGE ring gated on the last small input, so the
  smalls get the HBM port to themselves until ~10us and a2 then streams at
  the combined ~390 GB/s port rate. Scalar issues only the hot half of the
  weight blob (lands in parallel) and stays free for epilogues.
- the middle is PSUM-evacuation-throughput bound: every epilogue is a
  single fused 512-wide op, balanced across ACT and DVE.
- the final contraction consumes a2 chunk-by-chunk as each lands.
- a dense full-array warmup chain keeps the PE HAM clock gate open so the
  pipeline runs at 2.4GHz instead of 1.2.
"""

import numpy as np
from contextlib import ExitStack

import concourse.bass as bass
import concourse.tile as tile
from concourse import bacc, mybir
from concourse.bass_utils import run_bass_kernel_spmd

N_CORES = 8
N, E, G, H = 4096, 8192, 64, 128
NODE_DIM, EDGE_DIM = 11, 5
G_PER_CORE = G // N_CORES          # 8 graph slots per core
XD = NODE_DIM + 1                  # xs rows: w*x (11) + w (1)
ER = EDGE_DIM + 1                  # ea rows: ea (5) + ones (1)
XB = 32                            # xs/p1 base partition (operands need 0/32/64)

f32 = mybir.dt.float32
f16 = mybir.dt.float16
AF = mybir.ActivationFunctionType
OP = mybir.AluOpType

# wsmall column map (f16 weights packed into one [128, WSW] blob);
# the MLP-critical blocks (E1, P1, E2) lead: they are DMAed separately so
# their landing unblocks the interior matmuls
W_E1, W_P1, W_E2, W_E30, W_E31, W_P2, W_BR, W_B2 = (
    0, 128, 256, 512, 640, 768, 896, 1024)
W_E3B4 = 1152           # 512 cols: tile(e3_b, 4) on row 0 (matmul moving
                        # operands need base partition 0/32/64)
W_WME = 1664            # 12 cols: per-tile edge weights
W_E2B = 1676            # 2 cols: e2 bias halves (f16)
WSW = 1678
WSP = 512               # hot/cold split point of the wsmall DMA

NKCH = 8                           # a2 k-chunks (16 k's = 512KB each)
KPC = 128 // NKCH                  # 16 k's per chunk


def _slot_segments(caps, s):
    """(tile, p0, p1) segments of graph slot s in the (p, t) edge grid."""
    base = sum(caps[:s])
    segs, a, end = [], base, base + caps[s]
    while a < end:
        t, p0 = divmod(a, 128)
        take = min(128 - p0, end - a)
        segs.append((t, p0, p0 + take))
        a += take
    return segs


def _emit(nc, tc, io, caps):
    EP = sum(caps)
    NT = EP // 128
    # interior chunks of <=512 cols; edge-major groups of <=4 tiles
    chunks = []
    a = 0
    while a < EP:
        w = min(512, EP - a)
        chunks.append((a, w))
        a += w
    groups = []
    t = 0
    while t < NT:
        g = min(4, NT - t)
        groups.append((t, g))
        t += g

    es = ExitStack()
    const = es.enter_context(tc.tile_pool(name="const", bufs=1))
    a2pool = es.enter_context(tc.tile_pool(name="a2pool", bufs=1))
    big = es.enter_context(tc.tile_pool(name="big", bufs=1))
    work = es.enter_context(tc.tile_pool(name="work", bufs=4))
    psA = es.enter_context(tc.tile_pool(name="psA", bufs=4, space="PSUM"))
    psZ = es.enter_context(tc.tile_pool(name="psZ", bufs=3, space="PSUM"))
    psO = es.enter_context(tc.tile_pool(name="psO", bufs=1, space="PSUM"))

    with es:
        a2 = a2pool.tile([128, 128 * H], f16, tag="a2")
        wsmall = const.tile([128, WSW], f16, tag="wsmall")
        edge16 = const.tile([XB + XD, EP], f16, tag="edge16")

        # sync: edge16, cold weights, then even a2 chunks (FIFO keeps their
        # packets behind the smalls'); scalar: the hot weight half only
        # (lands in parallel, then the queue is free for epilogues);
        # gpsimd/SWDGE: odd a2 chunks gated on the last small input so Tile
        # cannot hoist them over it.
        nc.sync.dma_start(edge16[:], io["edge16"][:, :])
        nc.scalar.dma_start(wsmall[:, 0:WSP], io["wsmall"][:, 0:WSP])
        nc.sync.dma_start(wsmall[:, WSP:WSW], io["wsmall"][:, WSP:WSW])
        for c in range(NKCH):
            sl = slice(c * KPC * 128, (c + 1) * KPC * 128)
            nc.sync.dma_start(a2[:, sl], io["a2h"][:, sl])

        # dense warmup: one long full-array accumulation chain (no pool
        # waits, no has_written clears) keeps PE back-to-back busy from
        # kernel start so the HAM clock gate opens (~3.4us later) before
        # the real pipeline peaks. Output is garbage and never read.
        ones_r = const.tile([1, 128], f16, tag="ones_r")
        nc.vector.memset(ones_r[:], 1.0)
        ones_b = const.tile([128, 512], f16, tag="ones_b")
        nc.vector.memset(ones_b[:], 1.0)
        # touch the ACT function table now so the one-time ~1.3us table
        # load doesn't delay the first real epilogue
        scr = const.tile([1, 128], f16, tag="scr")
        nc.scalar.activation(scr[:], ones_r[:], AF.Relu)
        NWARM = 8
        pw = psA.tile([128, 512], f32, tag="mlp")
        for i in range(NWARM):
            nc.tensor.matmul(pw[:], ones_b[:, 0:128], ones_b[:],
                             start=(i == 0), stop=(i == NWARM - 1))

        # e2 bias halves f16 -> f32 for the ACT bias operand
        bias2 = const.tile([128, 2], f32, tag="bias2")
        nc.vector.tensor_copy(bias2[:], wsmall[:, W_E2B:W_E2B + 2])

        # ---- feature-major MLP interiors (single fused 512-wide epilogues
        # balanced across ACT and DVE) ----------------------------------------
        relu1 = big.tile([128, EP], f16, tag="relu1")
        e1o = big.tile([128, EP], f16, tag="e1o")
        e2o0 = big.tile([128, EP], f16, tag="e2o0")
        e2o1 = big.tile([128, EP], f16, tag="e2o1")
        for ci, (a, w) in enumerate(chunks):
            sl = slice(a, a + w)
            ps = psA.tile([128, 512], f32, tag="mlp")
            nc.tensor.matmul(ps[:, 0:w], wsmall[0:ER, W_E1:W_E1 + 128],
                             edge16[0:ER, sl], start=True, stop=True)
            if ci < 2:
                nc.scalar.activation(e1o[:, sl], ps[:, 0:w], AF.Relu)
            else:
                nc.vector.tensor_scalar_max(e1o[:, sl], ps[:, 0:w], 0.0)
        for a, w in chunks:
            sl = slice(a, a + w)
            ps2 = psA.tile([128, 512], f32, tag="mlp")
            nc.tensor.matmul(ps2[:, 0:w], wsmall[XB:XB + XD, W_P1:W_P1 + 128],
                             edge16[XB:XB + XD, sl], start=True, stop=True)
            nc.vector.tensor_scalar_max(relu1[:, sl], ps2[:, 0:w], 0.0)
        # broadcast e3b (tiled 4x) to all partitions for 512-wide adds
        pbc2 = psA.tile([128, 512], f32, tag="mlp")
        nc.tensor.matmul(pbc2[:], ones_r[:],
                         wsmall[0:1, W_E3B4:W_E3B4 + 512],
                         start=True, stop=True)
        e3bb = const.tile([128, 512], f32, tag="e3bb")
        nc.vector.tensor_copy(e3bb[:], pbc2[:])
        for a, w in chunks:
            for m, e2o in enumerate((e2o0, e2o1)):
                sl = slice(a, a + w)
                ps = psA.tile([128, 512], f32, tag="mlp")
                nc.tensor.matmul(ps[:, 0:w],
                                 wsmall[:, W_E2 + m * 128:W_E2 + (m + 1) * 128],
                                 e1o[:, sl], start=True, stop=True)
                if m == 0:
                    nc.scalar.activation(e2o[:, sl], ps[:, 0:w], AF.Relu,
                                         bias=bias2[:, 0:1])
                else:
                    nc.vector.tensor_scalar(e2o[:, sl], ps[:, 0:w],
                                            bias2[:, 1:2],
                                            0.0, op0=OP.add, op1=OP.max)

        # ---- edge-major last layers, 4-tile groups, 512-wide epilogues ------
        # h_big[e, t, i] = relu1[:, e].T @ p2w + p2b  (w_e, p1b folded on host)
        # e3x[e, t, k]   = relu(e2o[:, e].T @ e3w + e3b); col H = 1.0
        h_big = big.tile([128, NT, H], f16, tag="hbig")
        e3x = big.tile([128, NT, H + 1], f16, tag="e3x")
        nc.gpsimd.memset(e3x[:, :, H:H + 1], 1.0)
        for gi, (t0, gsz) in enumerate(groups):
            gw = gsz * 128
            psh = psA.tile([128, 512], f32, tag="mlp")
            pse = psA.tile([128, 512], f32, tag="mlp")
            for j in range(gsz):
                t = t0 + j
                sl = slice(t * 128, (t + 1) * 128)
                jj = slice(j * 128, (j + 1) * 128)
                nc.tensor.matmul(psh[:, jj], relu1[:, sl], wsmall[:, W_P2:W_P2 + 128],
                                 start=True, stop=True)
                nc.tensor.matmul(pse[:, jj], e2o0[:, sl], wsmall[:, W_E30:W_E30 + 128],
                                 start=True, stop=False)
                nc.tensor.matmul(pse[:, jj], e2o1[:, sl], wsmall[:, W_E31:W_E31 + 128],
                                 start=False, stop=True)
            g4s = slice(t0, t0 + gsz)
            if gi % 2 == 0:
                nc.scalar.copy(h_big[:, g4s, :], psh[:, 0:gw])
            else:
                nc.vector.tensor_copy(h_big[:, g4s, :], psh[:, 0:gw])
            t4 = work.tile([128, 512], f16, tag="t4")
            nc.vector.tensor_tensor(t4[:, 0:gw], pse[:, 0:gw], e3bb[:, 0:gw],
                                    op=OP.add)
            nc.scalar.activation(e3x[:, g4s, 0:H], t4[:, 0:gw], AF.Relu)

        # ---- per-graph ZG accumulation (single fp16 copy: zg values are
        # O(1) so fp16's 2^-11 relative error stays ~1e-3 of output scale,
        # well under the gate) ------------------------------------------------
        zg2 = big.tile([128, G_PER_CORE, H], f16, tag="zg2")
        # interleaved [hw_0 s_0 hw_1 s_1 ...]; strided stationary APs below
        hws = work.tile([128, 2 * G_PER_CORE], f16, tag="hws")
        for s in range(G_PER_CORE):
            segs = _slot_segments(caps, s)
            pz = psZ.tile([128, H + 2], f32, tag="zg")
            # the S chain starts only after the ZG chain closes: a matmul
            # with start=True clears has_written for its partitions across
            # the whole PSUM region, killing any open chain there
            for n, (t, p0, p1) in enumerate(segs):
                nc.tensor.matmul(pz[:, 0:H + 1], h_big[p0:p1, t, :],
                                 e3x[p0:p1, t, :],
                                 start=(n == 0), stop=(n == len(segs) - 1))
            for n, (t, p0, p1) in enumerate(segs):
                nc.tensor.matmul(pz[:, H + 1:H + 2], e3x[p0:p1, t, 0:H],
                                 wsmall[p0:p1, W_WME + t:W_WME + t + 1],
                                 start=(n == 0), stop=(n == len(segs) - 1))
            if s % 2 == 0:
                nc.scalar.copy(zg2[:, s, :], pz[:, 0:H])
            else:
                nc.vector.tensor_copy(zg2[:, s, :], pz[:, 0:H])
            nc.vector.tensor_copy(hws[:, 2 * s:2 * s + 2], pz[:, H:H + 2])
            # full-array filler matmuls: the ZG chains stall briefly on the
            # epilogue queues, and ~3.4us of reduced PE duty re-throttles
            # the HAM clock gate to 1.2GHz right before the contraction.
            # These keep the array busy through the gaps; output is junk.
            pf = psA.tile([128, 512], f32, tag="mlp")
            for df in range(2):
                nc.tensor.matmul(pf[:], ones_b[:, 0:128], ones_b[:],
                                 start=(df == 0), stop=(df == 1))

        # ---- final a2 contraction: a2 streams as the moving operand ----------
        # chunk-ordered so each group of 16 k's only waits on its own DMA
        # chunk; the e4-bias terms (hw @ Br, s @ B2) ride the group-0 chain
        # right after it opens
        po = psO.tile([128, 128], f32, tag="out")
        for c in range(NKCH):
            for r in range(KPC // 4):
                for j in range(4):
                    k = c * KPC + 4 * j + r
                    nc.tensor.matmul(po[32 * j:32 * j + 8, :], zg2[:, :, k],
                                     a2[:, k * 128:(k + 1) * 128],
                                     start=(c == 0 and r == 0),
                                     stop=(c == NKCH - 1 and r == KPC // 4 - 1),
                                     tile_position=(0, 32 * j))
            if c == 0:
                nc.tensor.matmul(po[0:G_PER_CORE, :], hws[:, 0:16:2],
                                 wsmall[:, W_BR:W_BR + 128],
                                 start=False, stop=False, tile_position=(0, 0))
                nc.tensor.matmul(po[0:G_PER_CORE, :], hws[:, 1:16:2],
                                 wsmall[:, W_B2:W_B2 + 128],
                                 start=False, stop=False, tile_position=(0, 0))

        ot = work.tile([128, 128], f32, tag="ot")
        nc.scalar.copy(ot[0:64, :], po[0:64, :])
        nc.vector.tensor_copy(ot[64:128, :], po[64:128, :])
        nc.sync.dma_start(io["pooled"][0:64, :], ot[0:64, :])
        nc.scalar.dma_start(io["pooled"][64:128, :], ot[64:128, :])


_CACHE = {}


def _build(caps):
    key = tuple(caps)
    if key in _CACHE:
        return _CACHE[key]
    EP = sum(caps)
    nc = bacc.Bacc("TRN2", target_bir_lowering=False, debug=False,
                   num_devices=N_CORES)
    io = {}

    def din(name, shape, dt=f32):
        io[name] = nc.dram_tensor(name, shape, dt, kind="ExternalInput").ap()

    din("edge16", [XB + XD, EP], f16)
    din("wsmall", [128, WSW], f16)
    din("a2h", [128, 128 * H], f16)
    io["pooled"] = nc.dram_tensor("pooled", [128, H], f32,
                                  kind="ExternalOutput").ap()

    with tile.TileContext(nc) as tc:
        _emit(nc, tc, io, caps)
    nc.compile()
    _CACHE[key] = nc
    return nc


def _host_prep(inputs):
    x = np.asarray(inputs["x"], dtype=np.float32)
    ea = np.asarray(inputs["edge_attr"], dtype=np.float32)
    ei = np.asarray(inputs["edge_index"]).astype(np.int64)
    batch = np.asarray(inputs["batch"]).astype(np.int64)
    src, dst = ei[0], ei[1]
    gid = batch[dst]
    cnt = np.bincount(dst, minlength=N).astype(np.float32)
    w_all = 1.0 / np.maximum(cnt, 1.0)

    # greedy graph->core balancing by 64-aligned edge count, slots sorted
    # largest-first so the shared per-slot capacity profile stays tight
    gcnt = np.bincount(gid, minlength=G)
    gcap = np.maximum((gcnt + 63) // 64 * 64, 64)
    order = np.argsort(-gcap, kind="stable")
    load = np.zeros(N_CORES, np.int64)
    nslot = np.zeros(N_CORES, np.int64)
    assign = np.empty((N_CORES, G_PER_CORE), np.int64)
    for g in order:
        c = min((c for c in range(N_CORES) if nslot[c] < G_PER_CORE),
                key=lambda c: (load[c], nslot[c]))
        assign[c, nslot[c]] = g
        load[c] += gcap[g]
        nslot[c] += 1
    caps = tuple(int(v) for v in gcap[assign].max(axis=0))
    # pad total to a multiple of 128 (whole tiles) on the last slot
    rem = -sum(caps) % 128
    caps = caps[:-1] + (caps[-1] + rem,)
    EP = sum(caps)
    offs = np.concatenate(([0], np.cumsum(caps)))
    NT = EP // 128

    e4w = np.asarray(inputs["e4_w"], np.float32).reshape(128, 128, 128)
    a2h = np.ascontiguousarray(
        e4w.transpose(1, 0, 2).reshape(128, 128 * H).astype(np.float16))
    p2b = np.asarray(inputs["p2_b"], np.float32)
    b2 = np.einsum("i,kio->ko", p2b, e4w).astype(np.float16)   # [k, o]
    br = np.asarray(inputs["e4_b"], np.float32).reshape(128, 128)
    br2 = p2b @ br                                             # [o]

    wsmall = np.zeros((128, WSW), np.float16)
    wsmall[:, W_P2:W_P2 + 128] = np.asarray(inputs["p2_w"], np.float16)
    wsmall[:, W_E2:W_E2 + 256] = np.asarray(inputs["e2_w"], np.float16)
    wsmall[:, W_E30:W_E30 + 128] = np.asarray(inputs["e3_w"], np.float16)[0:128]
    wsmall[:, W_E31:W_E31 + 128] = np.asarray(inputs["e3_w"], np.float16)[128:256]
    wsmall[:, W_BR:W_BR + 128] = br.astype(np.float16)
    wsmall[:, W_B2:W_B2 + 128] = b2
    wsmall[XB:XB + NODE_DIM, W_P1:W_P1 + 128] = np.asarray(inputs["p1_w"],
                                                           np.float16)
    wsmall[XB + NODE_DIM, W_P1:W_P1 + 128] = np.asarray(inputs["p1_b"],
                                                        np.float16)
    wsmall[0:EDGE_DIM, W_E1:W_E1 + 128] = np.asarray(inputs["e1_w"], np.float16)
    wsmall[EDGE_DIM, W_E1:W_E1 + 128] = np.asarray(inputs["e1_b"], np.float16)
    wsmall[0, W_E3B4:W_E3B4 + 512] = np.tile(
        np.asarray(inputs["e3_b"], np.float16), 4)
    wsmall[:, W_E2B:W_E2B + 2] = np.asarray(
        inputs["e2_b"], np.float16).reshape(2, 128).T

    in_maps = []
    wg_all = np.zeros((N_CORES, G_PER_CORE), np.float32)
    for c in range(N_CORES):
        ea_s = np.zeros((EP, EDGE_DIM), np.float32)
        xs_s = np.zeros((EP, XD), np.float32)
        w_s = np.zeros(EP, np.float32)
        for s in range(G_PER_CORE):
            es = np.where(gid == assign[c, s])[0]
            assert len(es) <= caps[s], f"slot {c},{s}: {len(es)} > {caps[s]}"
            pos = offs[s] + np.arange(len(es))
            we = w_all[dst[es]]
            ea_s[pos] = ea[es]
            xs_s[pos, 0:NODE_DIM] = x[src[es]] * we[:, None]
            xs_s[pos, NODE_DIM] = we
            w_s[pos] = we
            wg_all[c, s] = we.sum()

        edge16 = np.zeros((XB + XD, EP), np.float16)
        edge16[0:EDGE_DIM] = ea_s.T
        edge16[EDGE_DIM] = 1.0
        edge16[XB:XB + XD] = xs_s.T

        m = {"a2h": a2h}
        ws = wsmall.copy()
        ws[:, W_WME:W_WME + NT] = w_s.reshape(NT, 128).T.astype(np.float16)
        m["wsmall"] = np.ascontiguousarray(ws)
        m["edge16"] = np.ascontiguousarray(edge16)
        in_maps.append(m)
    return in_maps, wg_all, br2, caps, assign


def _run(inputs, trace=False, tmpdir=None):
    in_maps, wg_all, br2, caps, assign = _host_prep(inputs)
    nc = _build(caps)
    if trace:
        # No egress in this sandbox: neutralize the artifact upload the
        # trace path performs after NTFF capture, and register the NTFF
        # hook module if the image's antenv package lacks axon_hooks.
        from concourse import bass_utils as _bu
        _bu.upload_artifacts = lambda d: d
        try:
            from antenv import axon_hooks  # noqa: F401
        except ImportError:
            # This image's antenv lacks axon_hooks entirely: synthesize the
            # registry module and register the ctypes NTFF hook directly.
            import sys as _sys, types as _types
            import trn_agent_boot.trn_boot as _tb
            mod = _types.ModuleType("antenv.axon_hooks")
            mod._hook = _tb._ntff_profile_via_ctypes("/opt/axon/libaxon_pjrt.so")
            mod.set_axon_ntff_profile_hook = lambda h: setattr(mod, "_hook", h)
            mod.get_axon_ntff_profile_hook = lambda: mod._hook
            _sys.modules["antenv.axon_hooks"] = mod
            import antenv as _antenv
            _antenv.axon_hooks = mod
    res = run_bass_kernel_spmd(nc, in_maps, list(range(N_CORES)),
                               trace=trace, tmpdir=tmpdir)
    out = np.empty((G, H), np.float32)
    for c in range(N_CORES):
        p = res.results[c]["pooled"]
        acc = np.zeros((G_PER_CORE, H), np.float32)
        for j in range(4):
            acc += p[32 * j:32 * j + G_PER_CORE]
        acc += wg_all[c][:, None] * br2[None, :]   # W_g * (p2b @ Br)
        out[assign[c], :] = acc
    return out, res


def kernel(**inputs) -> np.ndarray:
    out, _ = _run(inputs)
    return out


# revision 26
# speedup vs baseline: 1.0458x; 1.0458x over previous
"""Trainium2 Bass kernel for nn_Net_16174846837292 (NNConv GNN message passing).

Strategy (graph-sharded, aggregation-folded, single fp16 a2 pass):
  pooled[g,o] = sum_{e: batch[dst[e]]=g} w_e * msg[e,o],  w_e = 1/max(cnt[dst_e],1)
  msg[e,o]    = sum_{k,i} e3[e,k]*h[src_e,i]*e4w[k,i*128+o] + sum_i h[src_e,i]*e4b[i*128+o]
  => pooled[g,o] = sum_k ZG_g[:,k]^T A2f[:,k*128+o] + HW_g^T Br
     ZG_g[i,k] = sum_{e in g} (w_e h[src_e,i]) e3[e,k],  HW_g[i] = sum_e w_e h[src_e,i]

Sharding: graphs are greedily re-balanced across the 8 cores by edge count
(the host owns the graph->core map and reassembles the output), 8 graph
slots per core sorted largest-first, so the shared compiled capacity
profile (max over cores per slot, 64-aligned) stays tight: EP ~1280 vs the
1536 a fixed worst-case cap needs. In-degree weights are per-edge host
constants and NO collectives are needed. The program is compiled per
capacity-profile (cached), so any input re-specializes correctly.

Schedule (from perfetto traces of the 38.4us baseline):
- the 4MB a2 stream starts at the first possible DMA slot, split into
  8 x 512KB chunks: evens on the sync HWDGE ring FIFO-behind the small
  inputs, odds on the SWDGE ring gated on the last small input, so the
  smalls get the HBM port to themselves until ~10us and a2 then streams at
  the combined ~390 GB/s port rate. Scalar issues only the hot half of the
  weight blob (lands in parallel) and stays free for epilogues.
- the middle is PSUM-evacuation-throughput bound: every epilogue is a
  single fused 512-wide op, balanced across ACT and DVE.
- the final contraction consumes a2 chunk-by-chunk as each lands.
- a dense full-array warmup chain keeps the PE HAM clock gate open so the
  pipeline runs at 2.4GHz instead of 1.2.
"""

import numpy as np
from contextlib import ExitStack

import concourse.bass as bass
import concourse.tile as tile
from concourse import bacc, mybir
from concourse.bass_utils import run_bass_kernel_spmd

N_CORES = 8
N, E, G, H = 4096, 8192, 64, 128
NODE_DIM, EDGE_DIM = 11, 5
G_PER_CORE = G // N_CORES          # 8 graph slots per core
XD = NODE_DIM + 1                  # xs rows: w*x (11) + w (1)
ER = EDGE_DIM + 1                  # ea rows: ea (5) + ones (1)
XB = 32                            # xs/p1 base partition (operands need 0/32/64)

f32 = mybir.dt.float32
f16 = mybir.dt.float16
AF = mybir.ActivationFunctionType
OP = mybir.AluOpType

# wsmall column map (f16 weights packed into one [128, WSW] blob);
# the MLP-critical blocks (E1, P1, E2) lead: they are DMAed separately so
# their landing unblocks the interior matmuls
W_E1, W_P1, W_E2, W_E30, W_E31, W_P2, W_BR, W_B2 = (
    0, 128, 256, 512, 640, 768, 896, 1024)
W_E3B4 = 1152           # 512 cols: tile(e3_b, 4) on row 0 (matmul moving
                        # operands need base partition 0/32/64)
W_WME = 1664            # 12 cols: per-tile edge weights
W_E2B = 1676            # 2 cols: e2 bias halves (f16)
WSW = 1678
WSP = 512               # hot/cold split point of the wsmall DMA

NKCH = 8                           # a2 k-chunks (16 k's = 512KB each)
KPC = 128 // NKCH                  # 16 k's per chunk


def _slot_segments(caps, s):
    """(tile, p0, p1) segments of graph slot s in the (p, t) edge grid."""
    base = sum(caps[:s])
    segs, a, end = [], base, base + caps[s]
    while a < end:
        t, p0 = divmod(a, 128)
        take = min(128 - p0, end - a)
        segs.append((t, p0, p0 + take))
        a += take
    return segs


def _emit(nc, tc, io, caps):
    EP = sum(caps)
    NT = EP // 128
    # interior chunks of <=512 cols; edge-major groups of <=4 tiles
    chunks = []
    a = 0
    while a < EP:
        w = min(512, EP - a)
        chunks.append((a, w))
        a += w
    groups = []
    t = 0
    while t < NT:
        g = min(4, NT - t)
        groups.append((t, g))
        t += g

    es = ExitStack()
    const = es.enter_context(tc.tile_pool(name="const", bufs=1))
    a2pool = es.enter_context(tc.tile_pool(name="a2pool", bufs=1))
    big = es.enter_context(tc.tile_pool(name="big", bufs=1))
    work = es.enter_context(tc.tile_pool(name="work", bufs=4))
    psA = es.enter_context(tc.tile_pool(name="psA", bufs=4, space="PSUM"))
    psZ = es.enter_context(tc.tile_pool(name="psZ", bufs=3, space="PSUM"))
    psO = es.enter_context(tc.tile_pool(name="psO", bufs=1, space="PSUM"))

    with es:
        a2 = a2pool.tile([128, 128 * H], f16, tag="a2")
        wsmall = const.tile([128, WSW], f16, tag="wsmall")
        edge16 = const.tile([XB + XD, EP], f16, tag="edge16")

        # sync: edge16, cold weights, then even a2 chunks (FIFO keeps their
        # packets behind the smalls'); scalar: the hot weight half only
        # (lands in parallel, then the queue is free for epilogues);
        # gpsimd/SWDGE: odd a2 chunks gated on the last small input so Tile
        # cannot hoist them over it.
        nc.sync.dma_start(edge16[:], io["edge16"][:, :])
        nc.scalar.dma_start(wsmall[:, 0:WSP], io["wsmall"][:, 0:WSP])
        nc.sync.dma_start(wsmall[:, WSP:WSW], io["wsmall"][:, WSP:WSW])
        for c in range(NKCH):
            sl = slice(c * KPC * 128, (c + 1) * KPC * 128)
            nc.sync.dma_start(a2[:, sl], io["a2h"][:, sl])

        # dense warmup: one long full-array accumulation chain (no pool
        # waits, no has_written clears) keeps PE back-to-back busy from
        # kernel start so the HAM clock gate opens (~3.4us later) before
        # the real pipeline peaks. Output is garbage and never read.
        ones_r = const.tile([1, 128], f16, tag="ones_r")
        nc.vector.memset(ones_r[:], 1.0)
        ones_b = const.tile([128, 512], f16, tag="ones_b")
        nc.vector.memset(ones_b[:], 1.0)
        # touch the ACT function table now so the one-time ~1.3us table
        # load doesn't delay the first real epilogue
        scr = const.tile([1, 128], f16, tag="scr")
        nc.scalar.activation(scr[:], ones_r[:], AF.Relu)
        NWARM = 6
        pw = psA.tile([128, 512], f32, tag="mlp")
        for i in range(NWARM):
            nc.tensor.matmul(pw[:], ones_b[:, 0:128], ones_b[:],
                             start=(i == 0), stop=(i == NWARM - 1))

        # e2 bias halves f16 -> f32 for the ACT bias operand
        bias2 = const.tile([128, 2], f32, tag="bias2")
        nc.vector.tensor_copy(bias2[:], wsmall[:, W_E2B:W_E2B + 2])

        # ---- feature-major MLP interiors (single fused 512-wide epilogues
        # balanced across ACT and DVE) ----------------------------------------
        relu1 = big.tile([128, EP], f16, tag="relu1")
        e1o = big.tile([128, EP], f16, tag="e1o")
        e2o0 = big.tile([128, EP], f16, tag="e2o0")
        e2o1 = big.tile([128, EP], f16, tag="e2o1")
        for ci, (a, w) in enumerate(chunks):
            sl = slice(a, a + w)
            ps = psA.tile([128, 512], f32, tag="mlp")
            nc.tensor.matmul(ps[:, 0:w], wsmall[0:ER, W_E1:W_E1 + 128],
                             edge16[0:ER, sl], start=True, stop=True)
            if ci < 2:
                nc.scalar.activation(e1o[:, sl], ps[:, 0:w], AF.Relu)
            else:
                nc.vector.tensor_scalar_max(e1o[:, sl], ps[:, 0:w], 0.0)
        for a, w in chunks:
            sl = slice(a, a + w)
            ps2 = psA.tile([128, 512], f32, tag="mlp")
            nc.tensor.matmul(ps2[:, 0:w], wsmall[XB:XB + XD, W_P1:W_P1 + 128],
                             edge16[XB:XB + XD, sl], start=True, stop=True)
            nc.vector.tensor_scalar_max(relu1[:, sl], ps2[:, 0:w], 0.0)
        # broadcast e3b (tiled 4x) to all partitions for 512-wide adds
        pbc2 = psA.tile([128, 512], f32, tag="mlp")
        nc.tensor.matmul(pbc2[:], ones_r[:],
                         wsmall[0:1, W_E3B4:W_E3B4 + 512],
                         start=True, stop=True)
        e3bb = const.tile([128, 512], f32, tag="e3bb")
        nc.vector.tensor_copy(e3bb[:], pbc2[:])
        for a, w in chunks:
            for m, e2o in enumerate((e2o0, e2o1)):
                sl = slice(a, a + w)
                ps = psA.tile([128, 512], f32, tag="mlp")
                nc.tensor.matmul(ps[:, 0:w],
                                 wsmall[:, W_E2 + m * 128:W_E2 + (m + 1) * 128],
                                 e1o[:, sl], start=True, stop=True)
                if m == 0:
                    nc.scalar.activation(e2o[:, sl], ps[:, 0:w], AF.Relu,
                                         bias=bias2[:, 0:1])
                else:
                    nc.vector.tensor_scalar(e2o[:, sl], ps[:, 0:w],
                                            bias2[:, 1:2],
                                            0.0, op0=OP.add, op1=OP.max)

        # ---- edge-major last layers, 4-tile groups, 512-wide epilogues ------
        # h_big[e, t, i] = relu1[:, e].T @ p2w + p2b  (w_e, p1b folded on host)
        # e3x[e, t, k]   = relu(e2o[:, e].T @ e3w + e3b); col H = 1.0
        h_big = big.tile([128, NT, H], f16, tag="hbig")
        e3x = big.tile([128, NT, H + 1], f16, tag="e3x")
        nc.gpsimd.memset(e3x[:, :, H:H + 1], 1.0)
        for gi, (t0, gsz) in enumerate(groups):
            gw = gsz * 128
            psh = psA.tile([128, 512], f32, tag="mlp")
            pse = psA.tile([128, 512], f32, tag="mlp")
            for j in range(gsz):
                t = t0 + j
                sl = slice(t * 128, (t + 1) * 128)
                jj = slice(j * 128, (j + 1) * 128)
                nc.tensor.matmul(psh[:, jj], relu1[:, sl], wsmall[:, W_P2:W_P2 + 128],
                                 start=True, stop=True)
                nc.tensor.matmul(pse[:, jj], e2o0[:, sl], wsmall[:, W_E30:W_E30 + 128],
                                 start=True, stop=False)
                nc.tensor.matmul(pse[:, jj], e2o1[:, sl], wsmall[:, W_E31:W_E31 + 128],
                                 start=False, stop=True)
            g4s = slice(t0, t0 + gsz)
            if gi % 2 == 0:
                nc.scalar.copy(h_big[:, g4s, :], psh[:, 0:gw])
            else:
                nc.vector.tensor_copy(h_big[:, g4s, :], psh[:, 0:gw])
            t4 = work.tile([128, 512], f16, tag="t4")
            nc.vector.tensor_tensor(t4[:, 0:gw], pse[:, 0:gw], e3bb[:, 0:gw],
                                    op=OP.add)
            nc.scalar.activation(e3x[:, g4s, 0:H], t4[:, 0:gw], AF.Relu)

        # ---- per-graph ZG accumulation (single fp16 copy: zg values are
        # O(1) so fp16's 2^-11 relative error stays ~1e-3 of output scale,
        # well under the gate) ------------------------------------------------
        zg2 = big.tile([128, G_PER_CORE, H], f16, tag="zg2")
        # interleaved [hw_0 s_0 hw_1 s_1 ...]; strided stationary APs below
        hws = work.tile([128, 2 * G_PER_CORE], f16, tag="hws")
        for s in range(G_PER_CORE):
            segs = _slot_segments(caps, s)
            pz = psZ.tile([128, H + 2], f32, tag="zg")
            # the S chain starts only after the ZG chain closes: a matmul
            # with start=True clears has_written for its partitions across
            # the whole PSUM region, killing any open chain there
            for n, (t, p0, p1) in enumerate(segs):
                nc.tensor.matmul(pz[:, 0:H + 1], h_big[p0:p1, t, :],
                                 e3x[p0:p1, t, :],
                                 start=(n == 0), stop=(n == len(segs) - 1))
            for n, (t, p0, p1) in enumerate(segs):
                nc.tensor.matmul(pz[:, H + 1:H + 2], e3x[p0:p1, t, 0:H],
                                 wsmall[p0:p1, W_WME + t:W_WME + t + 1],
                                 start=(n == 0), stop=(n == len(segs) - 1))
            if s % 2 == 0:
                nc.scalar.copy(zg2[:, s, :], pz[:, 0:H])
            else:
                nc.vector.tensor_copy(zg2[:, s, :], pz[:, 0:H])
            nc.vector.tensor_copy(hws[:, 2 * s:2 * s + 2], pz[:, H:H + 2])
            # full-array filler matmuls: the ZG chains stall briefly on the
            # epilogue queues, and ~3.4us of reduced PE duty re-throttles
            # the HAM clock gate to 1.2GHz right before the contraction.
            # These keep the array busy through the gaps; output is junk.
            if s in (3, 5, 7):
                pf = psA.tile([128, 512], f32, tag="mlp")
                for df in range(2):
                    nc.tensor.matmul(pf[:], ones_b[:, 0:128], ones_b[:],
                                     start=(df == 0), stop=(df == 1))

        # ---- final a2 contraction: a2 streams as the moving operand ----------
        # chunk-ordered so each group of 16 k's only waits on its own DMA
        # chunk; the e4-bias terms (hw @ Br, s @ B2) ride the group-0 chain
        # right after it opens
        po = psO.tile([128, 128], f32, tag="out")
        for c in range(NKCH):
            for r in range(KPC // 4):
                for j in range(4):
                    k = c * KPC + 4 * j + r
                    nc.tensor.matmul(po[32 * j:32 * j + 8, :], zg2[:, :, k],
                                     a2[:, k * 128:(k + 1) * 128],
                                     start=(c == 0 and r == 0),
                                     stop=(c == NKCH - 1 and r == KPC // 4 - 1),
                                     tile_position=(0, 32 * j))
            if c == 0:
                nc.tensor.matmul(po[0:G_PER_CORE, :], hws[:, 0:16:2],
                                 wsmall[:, W_BR:W_BR + 128],
                                 start=False, stop=False, tile_position=(0, 0))
                nc.tensor.matmul(po[0:G_PER_CORE, :], hws[:, 1:16:2],
                                 wsmall[:, W_B2:W_B2 + 128],
                                 start=False, stop=False, tile_position=(0, 0))

        ot = work.tile([128, 128], f32, tag="ot")
        nc.scalar.copy(ot[0:64, :], po[0:64, :])
        nc.vector.tensor_copy(ot[64:128, :], po[64:128, :])
        nc.sync.dma_start(io["pooled"][0:64, :], ot[0:64, :])
        nc.scalar.dma_start(io["pooled"][64:128, :], ot[64:128, :])


_CACHE = {}


def _build(caps):
    key = tuple(caps)
    if key in _CACHE:
        return _CACHE[key]
    EP = sum(caps)
    nc = bacc.Bacc("TRN2", target_bir_lowering=False, debug=False,
                   num_devices=N_CORES)
    io = {}

    def din(name, shape, dt=f32):
        io[name] = nc.dram_tensor(name, shape, dt, kind="ExternalInput").ap()

    din("edge16", [XB + XD, EP], f16)
    din("wsmall", [128, WSW], f16)
    din("a2h", [128, 128 * H], f16)
    io["pooled"] = nc.dram_tensor("pooled", [128, H], f32,
                                  kind="ExternalOutput").ap()

    with tile.TileContext(nc) as tc:
        _emit(nc, tc, io, caps)
    nc.compile()
    _CACHE[key] = nc
    return nc


def _host_prep(inputs):
    x = np.asarray(inputs["x"], dtype=np.float32)
    ea = np.asarray(inputs["edge_attr"], dtype=np.float32)
    ei = np.asarray(inputs["edge_index"]).astype(np.int64)
    batch = np.asarray(inputs["batch"]).astype(np.int64)
    src, dst = ei[0], ei[1]
    gid = batch[dst]
    cnt = np.bincount(dst, minlength=N).astype(np.float32)
    w_all = 1.0 / np.maximum(cnt, 1.0)

    # greedy graph->core balancing by 64-aligned edge count, slots sorted
    # largest-first so the shared per-slot capacity profile stays tight
    gcnt = np.bincount(gid, minlength=G)
    gcap = np.maximum((gcnt + 63) // 64 * 64, 64)
    order = np.argsort(-gcap, kind="stable")
    load = np.zeros(N_CORES, np.int64)
    nslot = np.zeros(N_CORES, np.int64)
    assign = np.empty((N_CORES, G_PER_CORE), np.int64)
    for g in order:
        c = min((c for c in range(N_CORES) if nslot[c] < G_PER_CORE),
                key=lambda c: (load[c], nslot[c]))
        assign[c, nslot[c]] = g
        load[c] += gcap[g]
        nslot[c] += 1
    caps = tuple(int(v) for v in gcap[assign].max(axis=0))
    # pad total to a multiple of 128 (whole tiles) on the last slot
    rem = -sum(caps) % 128
    caps = caps[:-1] + (caps[-1] + rem,)
    EP = sum(caps)
    offs = np.concatenate(([0], np.cumsum(caps)))
    NT = EP // 128

    e4w = np.asarray(inputs["e4_w"], np.float32).reshape(128, 128, 128)
    a2h = np.ascontiguousarray(
        e4w.transpose(1, 0, 2).reshape(128, 128 * H).astype(np.float16))
    p2b = np.asarray(inputs["p2_b"], np.float32)
    b2 = np.einsum("i,kio->ko", p2b, e4w).astype(np.float16)   # [k, o]
    br = np.asarray(inputs["e4_b"], np.float32).reshape(128, 128)
    br2 = p2b @ br                                             # [o]

    wsmall = np.zeros((128, WSW), np.float16)
    wsmall[:, W_P2:W_P2 + 128] = np.asarray(inputs["p2_w"], np.float16)
    wsmall[:, W_E2:W_E2 + 256] = np.asarray(inputs["e2_w"], np.float16)
    wsmall[:, W_E30:W_E30 + 128] = np.asarray(inputs["e3_w"], np.float16)[0:128]
    wsmall[:, W_E31:W_E31 + 128] = np.asarray(inputs["e3_w"], np.float16)[128:256]
    wsmall[:, W_BR:W_BR + 128] = br.astype(np.float16)
    wsmall[:, W_B2:W_B2 + 128] = b2
    wsmall[XB:XB + NODE_DIM, W_P1:W_P1 + 128] = np.asarray(inputs["p1_w"],
                                                           np.float16)
    wsmall[XB + NODE_DIM, W_P1:W_P1 + 128] = np.asarray(inputs["p1_b"],
                                                        np.float16)
    wsmall[0:EDGE_DIM, W_E1:W_E1 + 128] = np.asarray(inputs["e1_w"], np.float16)
    wsmall[EDGE_DIM, W_E1:W_E1 + 128] = np.asarray(inputs["e1_b"], np.float16)
    wsmall[0, W_E3B4:W_E3B4 + 512] = np.tile(
        np.asarray(inputs["e3_b"], np.float16), 4)
    wsmall[:, W_E2B:W_E2B + 2] = np.asarray(
        inputs["e2_b"], np.float16).reshape(2, 128).T

    in_maps = []
    wg_all = np.zeros((N_CORES, G_PER_CORE), np.float32)
    for c in range(N_CORES):
        ea_s = np.zeros((EP, EDGE_DIM), np.float32)
        xs_s = np.zeros((EP, XD), np.float32)
        w_s = np.zeros(EP, np.float32)
        for s in range(G_PER_CORE):
            es = np.where(gid == assign[c, s])[0]
            assert len(es) <= caps[s], f"slot {c},{s}: {len(es)} > {caps[s]}"
            pos = offs[s] + np.arange(len(es))
            we = w_all[dst[es]]
            ea_s[pos] = ea[es]
            xs_s[pos, 0:NODE_DIM] = x[src[es]] * we[:, None]
            xs_s[pos, NODE_DIM] = we
            w_s[pos] = we
            wg_all[c, s] = we.sum()

        edge16 = np.zeros((XB + XD, EP), np.float16)
        edge16[0:EDGE_DIM] = ea_s.T
        edge16[EDGE_DIM] = 1.0
        edge16[XB:XB + XD] = xs_s.T

        m = {"a2h": a2h}
        ws = wsmall.copy()
        ws[:, W_WME:W_WME + NT] = w_s.reshape(NT, 128).T.astype(np.float16)
        m["wsmall"] = np.ascontiguousarray(ws)
        m["edge16"] = np.ascontiguousarray(edge16)
        in_maps.append(m)
    return in_maps, wg_all, br2, caps, assign


def _run(inputs, trace=False, tmpdir=None):
    in_maps, wg_all, br2, caps, assign = _host_prep(inputs)
    nc = _build(caps)
    if trace:
        # No egress in this sandbox: neutralize the artifact upload the
        # trace path performs after NTFF capture, and register the NTFF
        # hook module if the image's antenv package lacks axon_hooks.
        from concourse import bass_utils as _bu
        _bu.upload_artifacts = lambda d: d
        try:
            from antenv import axon_hooks  # noqa: F401
        except ImportError:
            # This image's antenv lacks axon_hooks entirely: synthesize the
            # registry module and register the ctypes NTFF hook directly.
            import sys as _sys, types as _types
            import trn_agent_boot.trn_boot as _tb
            mod = _types.ModuleType("antenv.axon_hooks")
            mod._hook = _tb._ntff_profile_via_ctypes("/opt/axon/libaxon_pjrt.so")
            mod.set_axon_ntff_profile_hook = lambda h: setattr(mod, "_hook", h)
            mod.get_axon_ntff_profile_hook = lambda: mod._hook
            _sys.modules["antenv.axon_hooks"] = mod
            import antenv as _antenv
            _antenv.axon_hooks = mod
    res = run_bass_kernel_spmd(nc, in_maps, list(range(N_CORES)),
                               trace=trace, tmpdir=tmpdir)
    out = np.empty((G, H), np.float32)
    for c in range(N_CORES):
        p = res.results[c]["pooled"]
        acc = np.zeros((G_PER_CORE, H), np.float32)
        for j in range(4):
            acc += p[32 * j:32 * j + G_PER_CORE]
        acc += wg_all[c][:, None] * br2[None, :]   # W_g * (p2b @ Br)
        out[assign[c], :] = acc
    return out, res


def kernel(**inputs) -> np.ndarray:
    out, _ = _run(inputs)
    return out
